# revision 36
# baseline (speedup 1.0000x reference)
"""Trainium2 Bass kernel for per-sample dynamic (CDNA) depthwise 5x5 conv.

Computation (per sample b):
  k = relu(emb_flat @ W.T + b - 1e-5) + 1e-5        [225] -> [9, 25]
  k = k / k.sum(-1, keepdims=True)                  normalized 5x5 kernels
  out[k,c,h,w] = sum_{i,j} k[k,5i+j] * pad(rgb)[c,h+i,w+j]   [9,3,256,256]

Sharding: data-parallel over batch, 4 samples per core on 8 cores.

Conv-as-matmul mapping, single-stream variant: all 25 taps live in the
contraction dim.  For an output row-tile of HH=14 rows the lhsT is a
banded [90, 128] matrix with partition p = r*5 + j (r = input row within
the 18-row strip, j = horizontal tap) and
  lhsT[r*5+j, hh*9+k] = kn[k, 5*(r-hh)+j] / Z[k]   for 0 <= r-hh <= 4.
The rhs [90, N] holds the input strip replicated 5x with horizontal
shifts: rhs[r*5+j, (c,w)] = padded[c, h0+r, w+j].  One matmul per
(sample, psum-bank-chunk) computes 126 output rows (9 kernels x 14 image
rows) in a single stream -- 5x fewer PE cycles than accumulating the 5
horizontal taps.  The replicated rhs is pre-materialized host-side so
each tile needs exactly one big contiguous HBM load.

Everything runs in bf16 (inputs, weights, output) with f32 PSUM
accumulation; the kernel normalization 1/Z is folded into the banded
weights so PSUM evacuation is a pure copy/cast, split across the DVE,
Activation and GpSimd engines.  Output rows are written h-major so each
(tile, sample-pair) evacuation is one strided DMA; the host transposes
[B,H,K,C,W] -> [B,K,C,H,W] at the end.
"""

import sys
import numpy as np

try:
    import concourse  # noqa: F401
except ImportError:
    sys.path.insert(0, "/opt/trn_rl_repo")

import ml_dtypes

BF16 = ml_dtypes.bfloat16

KER = 5
NK = 9
SHIFT = 1e-5
B, C, H, W_IMG = 32, 3, 256, 256
PAD = KER // 2
HPH = H + 2 * PAD           # 260 padded rows
ROWW = W_IMG + 2 * PAD      # 260 useful row width
WPAD = W_IMG + 2 * PAD + 4  # 264 host pad width (shift overflow room)
NCORES = 8
BL = B // NCORES            # 4 samples per core
FCIN = 8192
FCOUT = NK * KER * KER      # 225
HH = 14                     # output rows per conv tile
M_REAL = NK * HH            # 126
MPAD = 128                  # padded lhsT free size (FWL wants 128)
KR = (HH + KER - 1) * KER   # 90 contraction partitions (r*5+j)
NT = 18 + 1                 # 18 full tiles + one overlapping tail tile
H0S = [14 * t for t in range(18)] + [H - HH]  # last tile at 242
TAIL_HH0 = 10               # tail tile only writes hh >= 10 (h 252..255)
NCHUNK = FCIN // 128        # 64

CW = C * ROWW               # 780 free elems per (sample, strip-row)
OUT_HSTRIDE = NK * C * W_IMG    # 6912 elems per output row h
OUT_BSTRIDE = H * OUT_HSTRIDE   # 1769472 elems per sample

_CACHE = {}


def _build_nc():
    import concourse.bass as bass
    import concourse.bacc as bacc
    import concourse.mybir as mybir
    from concourse import tile
    from contextlib import ExitStack

    f32 = mybir.dt.float32
    bf16 = mybir.dt.bfloat16
    AF = mybir.ActivationFunctionType
    ALU = mybir.AluOpType

    nc = bacc.Bacc("TRN2", target_bir_lowering=False, debug=False)

    # per-core external inputs.  wt/embt come pre-swizzled host-side so the
    # SBUF load is one contiguous run per partition (128 descriptors, not
    # 8192): wt2[p, c, n] = W.T[c*128+p, n].
    rgbrep = nc.dram_tensor("rgbrep", [NT, KR, BL * CW], bf16,
                            kind="ExternalInput")
    wt = nc.dram_tensor("wt", [128, NCHUNK * FCOUT], bf16,
                        kind="ExternalInput")
    embt = nc.dram_tensor("embt", [128, NCHUNK * BL], bf16,
                          kind="ExternalInput")
    biasbc = nc.dram_tensor("biasbc", [BL, FCOUT], f32, kind="ExternalInput")
    # raw dump of the per-tile output staging tiles; host reassembles
    out2 = nc.dram_tensor("out2", [NT, M_REAL, 2 * 2 * C * W_IMG], bf16,
                          kind="ExternalOutput")

    maskb = nc.dram_tensor("maskb", [KR, MPAD], bf16, kind="ExternalInput")
    # DRAM scratch: compact permuted kernels, padded so the banded gather's
    # out-of-band reads stay in-bounds (masked to zero afterwards)
    KOFF = 640
    knpd = nc.dram_tensor("knpd", [3072], bf16)  # = KOFF + 900 + tail pad

    with tile.TileContext(nc) as tc, ExitStack() as ctx:
        setup = ctx.enter_context(tc.tile_pool(name="setup", bufs=1))
        persist = ctx.enter_context(tc.tile_pool(name="persist", bufs=1))
        rep_pool = ctx.enter_context(tc.tile_pool(name="rep", bufs=4))
        osb_pool = ctx.enter_context(tc.tile_pool(name="osb", bufs=4))

        # ---------------- FC (b-major: M=4, N=225) ----------------
        # small FC inputs first (they gate the first FC matmul), then the wt
        # chunks split across both HWDGE rings.
        embt_sb = setup.tile([128, NCHUNK * BL], bf16, tag="embt")
        nc.sync.dma_start(embt_sb[:], embt.ap())
        bias_sb = setup.tile([BL, FCOUT], f32, tag="bias")
        nc.scalar.dma_start(bias_sb[:], biasbc.ap())
        mask_sb = setup.tile([KR, MPAD], bf16, tag="mask")
        nc.scalar.dma_start(mask_sb[:], maskb.ap())
        NWC = 6
        CPW = 11  # fc-chunks per wt tile (6*11 > 64; last tile smaller)
        wt_engines = [nc.sync, nc.scalar, nc.gpsimd]
        wt_a = []
        for wi in range(NWC):
            cw = min(CPW, NCHUNK - wi * CPW)
            wtile = setup.tile([128, cw * FCOUT], bf16, tag=f"wt{wi}")
            wt_engines[wi % 3].dma_start(
                wtile[:], bass.AP(wt, wi * CPW * FCOUT,
                                  [[NCHUNK * FCOUT, 128], [1, cw * FCOUT]]))
            wt_a.append(wtile)

        with tc.tile_pool(name="psum_fc", bufs=1, space="PSUM") as psum_fc:
            kfc = psum_fc.tile([BL, FCOUT], f32, tag="kfc")
            for ci in range(NCHUNK):
                nc.tensor.matmul(
                    kfc[:],
                    lhsT=embt_sb[:, ci * BL:(ci + 1) * BL],
                    rhs=wt_a[ci // CPW][:, (ci % CPW) * FCOUT:
                                        (ci % CPW + 1) * FCOUT],
                    start=(ci == 0),
                    stop=(ci == NCHUNK - 1),
                )

            # knr = relu(kfc + (bias - shift)) + shift, then fold in 1/Z
            knr = setup.tile([BL, FCOUT], f32, tag="knr")
            nc.vector.tensor_tensor(knr[:], kfc[:], bias_sb[:], op=ALU.add)
        nc.vector.tensor_scalar(knr[:], knr[:], 0.0, SHIFT,
                                op0=ALU.max, op1=ALU.add)
        zs = setup.tile([BL, NK], f32, tag="zs")
        nc.vector.reduce_sum(
            zs[:], knr[:].rearrange("b (k p) -> b k p", k=NK),
            axis=mybir.AxisListType.X,
        )
        zr = setup.tile([BL, NK], f32, tag="zr")
        nc.vector.reciprocal(zr[:], zs[:])
        knd_sb = setup.tile([BL, FCOUT], f32, tag="knd")
        nc.vector.tensor_tensor(
            knd_sb[:].rearrange("b (k p) -> b k p", k=NK),
            knr[:].rearrange("b (k p) -> b k p", k=NK),
            zr[:].unsqueeze(2).broadcast_to([BL, NK, KER * KER]),
            op=ALU.mult,
        )

        # permute fc -> (d, j, k) within the free dim + cast to bf16
        knp = setup.tile([BL, FCOUT], bf16, tag="knp")
        nc.vector.tensor_copy(
            knp[:].rearrange("b (d j k) -> b d j k", d=KER, j=KER),
            knd_sb[:].rearrange("b (k d j) -> b d j k", k=NK, d=KER),
        )

        # zero-fill the knpd pad regions (off the critical chain), then
        # store the compact kernels in the middle
        zt2 = setup.tile([128, 24], bf16, tag="zt2")
        nc.vector.memset(zt2[:], 0.0)
        nc.scalar.dma_start(
            bass.AP(knpd, 0, [[24, 128], [1, 24]]), zt2[:])
        nc.sync.dma_start(
            bass.AP(knpd, KOFF, [[FCOUT, BL], [1, FCOUT]]), knp[:])

        # banded lhsT via one windowed load + one fused strided mask-multiply.
        # hh runs REVERSED in the output rows (m = (13-hh)*9 + k) so all view
        # strides stay positive:
        #   lhsT[p, b, hh'*9+k] = knpd[KOFF-585 + 9p + 225b + 45hh' + 9k]
        #                       = win[p, 225b + 45hh' + 9k],  masked in-band.
        WINW = 1344
        win = persist.tile([KR, WINW], bf16, tag="win")
        nc.sync.dma_start(
            win[:], bass.AP(knpd, KOFF - 585, [[NK, KR], [1, WINW]]))
        lhsT = persist.tile([KR, BL * MPAD], bf16, tag="lhsT")
        nc.vector.memset(lhsT[:], 0.0)
        nc.vector.tensor_tensor(
            bass.AP(lhsT[:].tensor, 0,
                    [[BL * MPAD, KR], [MPAD, BL], [NK, HH], [1, NK]]),
            bass.AP(win[:].tensor, 0,
                    [[WINW, KR], [FCOUT, BL], [KER * NK, HH], [1, NK]]),
            bass.AP(mask_sb[:].tensor, 0,
                    [[MPAD, KR], [0, BL], [NK, HH], [1, NK]]),
            op=ALU.mult,
        )

        # ---------------- conv main loop ----------------
        def evac_dve(dst, src):
            nc.vector.tensor_copy(dst, src)

        def evac_act(dst, src):
            nc.scalar.activation(dst, src, AF.Copy)

        # GPSIMD cannot read PSUM; alternate evacuation DVE/Act
        evac_engines = [evac_dve, evac_act]
        evac_i = 0
        with tc.tile_pool(name="psum_conv", bufs=2, space="PSUM") as psc:
            for t in range(NT):
                rep = rep_pool.tile([KR, BL * CW], bf16, tag="rep")
                nc.gpsimd.dma_start(rep[:], rgbrep.ap()[t])
                rv = rep[:].rearrange("p (b c w) -> p b c w", b=BL, c=C)
                osb = osb_pool.tile([MPAD, BL * C * W_IMG], bf16, tag="osb")
                for sp in range(2):
                    for bl in range(2):
                        b = 2 * sp + bl
                        ps = psc.tile([MPAD, C * W_IMG], f32, tag=f"ps{bl}")
                        lt = lhsT[:, b * MPAD:(b + 1) * MPAD]
                        nc.tensor.matmul(
                            ps[:, 0:2 * W_IMG], lhsT=lt,
                            rhs=rv[:, b, 0:2, 0:W_IMG],
                            start=True, stop=True,
                        )
                        nc.tensor.matmul(
                            ps[:, 2 * W_IMG:C * W_IMG], lhsT=lt,
                            rhs=rv[:, b, 2, 0:W_IMG],
                            start=True, stop=True,
                        )
                        eng = evac_engines[evac_i % 2]
                        evac_i += 1
                        eng(osb[:, b * C * W_IMG:(b + 1) * C * W_IMG],
                            ps[:])
                # one contiguous dump per tile (126 x 6KB descriptors),
                # alternating between the two HWDGE rings; the tail tile
                # only has 36 fresh rows (hh'=0..3)
                out_eng = nc.sync if t % 2 == 0 else nc.scalar
                if t < NT - 1:
                    out_eng.dma_start(out2.ap()[t], osb[0:M_REAL, :])
                else:
                    nrow = (HH - TAIL_HH0) * NK
                    out_eng.dma_start(
                        bass.AP(out2, t * M_REAL * BL * C * W_IMG,
                                [[BL * C * W_IMG, nrow], [1, BL * C * W_IMG]]),
                        osb[0:nrow, :])
    nc.compile()
    return nc


def _host_prep(emb, rgb, W, b):
    # wt2[p, c, n] = W.T[c*128+p, n]; embt2[p, c, b] = emb_flat[c*128+p, b]
    # -> the SBUF load is one contiguous 28.8KB/0.5KB run per partition.
    wt2 = np.ascontiguousarray(
        W.T.astype(BF16).reshape(NCHUNK, 128, FCOUT).transpose(1, 0, 2)
    ).reshape(128, NCHUNK * FCOUT)
    # band mask (hh reversed): maskb[p, hh'*9+k] = 1 iff
    # 0 <= p//5 - (13-hh') <= 4
    maskb = np.zeros((KR, MPAD), dtype=BF16)
    for p in range(KR):
        for hp in range(HH):
            if 0 <= p // KER - (HH - 1 - hp) <= KER - 1:
                maskb[p, hp * NK:(hp + 1) * NK] = 1
    emb_t = emb.reshape(B, FCIN).T.astype(BF16)          # [8192, 32]
    biasbc = np.broadcast_to((b.astype(np.float32) - SHIFT)[None, :],
                             (BL, FCOUT)).copy()

    # replicated+shifted conv rhs: rep[t, r*5+j, b, c*260+w] =
    #   padded[b, c, h0[t]+r, w+j]
    padded = np.pad(rgb, ((0, 0), (0, 0), (PAD, PAD),
                          (PAD, PAD + 4))).astype(BF16)  # [32,3,260,264]
    sw = np.lib.stride_tricks.sliding_window_view(
        padded, ROWW, axis=3)                            # [32,3,260,5,260]
    idx = np.asarray(H0S)[:, None] + np.arange(HH + KER - 1)[None, :]
    g = sw[:, :, idx]                                    # [32,3,19,18,5,260]
    repf = np.ascontiguousarray(
        g.transpose(2, 3, 4, 0, 1, 5)).reshape(NT, KR, B, CW)

    in_maps = []
    for core in range(NCORES):
        sl = slice(core * BL, (core + 1) * BL)
        embt2 = np.ascontiguousarray(
            emb_t[:, sl].reshape(NCHUNK, 128, BL).transpose(1, 0, 2)
        ).reshape(128, NCHUNK * BL)
        in_maps.append({
            "rgbrep": np.ascontiguousarray(repf[:, :, sl]).reshape(
                NT, KR, BL * CW),
            "wt": wt2,
            "embt": embt2,
            "biasbc": biasbc,
            "maskb": maskb,
        })
    return in_maps


def _assemble(raw_outs):
    """raw_outs: per-core [NT, M_REAL, BL*C*W] bf16 dumps -> [B,K,C,H,W] f32."""
    full = np.empty((B, NK, C, H, W_IMG), dtype=np.float32)
    for core, o in enumerate(raw_outs):
        # [t, (hh' k), (b c w)] -> [t, hh, k, b, c, w]; hh' = 13-hh
        o = np.asarray(o).reshape(NT, HH, NK, BL, C, W_IMG)[:, ::-1]
        sl = slice(core * BL, (core + 1) * BL)
        v = o.transpose(0, 3, 2, 4, 1, 5)        # [t, b, k, c, hh, w]
        for t in range(NT - 1):
            full[sl, :, :, H0S[t]:H0S[t] + HH, :] = v[t]
        full[sl, :, :, H - (HH - TAIL_HH0):, :] = v[NT - 1][:, :, :,
                                                           TAIL_HH0:, :]
    return full


def get_nc():
    if "nc" not in _CACHE:
        _CACHE["nc"] = _build_nc()
    return _CACHE["nc"]


def kernel(emb, rgb, W, b):
    from concourse.bass_utils import run_bass_kernel_spmd

    emb = np.asarray(emb, dtype=np.float32)
    rgb = np.asarray(rgb, dtype=np.float32)
    W = np.asarray(W, dtype=np.float32)
    b = np.asarray(b, dtype=np.float32)
    assert emb.shape == (B, 128, 8, 8) and rgb.shape == (B, C, H, W_IMG)

    nc = get_nc()
    in_maps = _host_prep(emb, rgb, W, b)
    res = run_bass_kernel_spmd(nc, in_maps, list(range(NCORES)))
    return _assemble([r["out2"] for r in res.results])
